# revision 40
# baseline (speedup 1.0000x reference)
"""AttentionPooling Trainium2 kernel (8-core data-parallel SPMD).

Reference computation per batch b (B=2048, T=200, E=H=64):
    att_in = [q, k, q-k, q*k]            (T, 4E)
    h   = elu(att_in @ W1 + b1)          (T, H)
    s   = h @ W2 + b2                    (T,)
    s   = where(mask, s, PAD); p = softmax(s)
    out = p @ k                          (E,)

Algebraic restructuring used here:
  att_in @ W1 = q@(W1a+W1c) + k@(W1b-W1c) + (q*k)@W1d
              = c(q)  +  k @ Wk  +  (q*k) @ Wp          [c is per-batch row]
  For a subset of pairs (NF_FOLD) the q*k product is folded into a
  per-batch stationary on the host:  k@Wk + (q*k)@Wp = k @ (Wk+diag(q)Wp),
  halving those pairs' PE score work and removing their q*k elementwise
  product entirely (at the cost of streaming the per-batch stationaries).
  elu(x)+1 = relu(x) + min(exp(x), 1)   (exact), and softmax is invariant
  to additive constants, so the +1 and b2 drop out of the softmax.
  Softmax uses no max-shift (scores are O(5) for this data distribution;
  exp stays comfortably finite in fp32) and folds masking in post-exp:
  p = (exp(s)*m) / sum(exp(s)*m).

Device layout: batches processed in pairs (2 batches span the 128
partitions: partition = 64*pb + e).  16 pairs form a group of 32 batches.
Host pre-packs keys twice (transposed bf16 for the score branch, natural
bf16 for the weighted sum) so no on-chip transpose of the big tensor is
needed.  Scores for all 16 pairs of a group accumulate into one [32, 200]
PSUM tile via zero-padded replicated-W2 stationaries.

Engine budget tuning (cost-model driven):
  - remaining q*k products on Pool (SBUF-only engine; cannot touch PSUM)
  - exp() output bf16 so the min() runs in DVE 4x mode
  - relu+add: scalar_tensor_tensor on DVE (PSUM read) for half the
    blocks; ACT Relu + DVE 2x add for the other half
  - softmax mask-mult + row-sum fused in one tensor_tensor_reduce
  - PSUM->SBUF staging copies on DVE, freeing ACT for the big exp()s
"""

import os
import sys

import numpy as np

sys.path.insert(0, "/opt/trn_rl_repo")

import ml_dtypes

B, T, E, H = 2048, 200, 64, 64
NCORES = 8
BC = B // NCORES  # 256 batches per core
NPG = 16          # pairs per group
GB = 2 * NPG      # 32 batches per group
G = BC // GB      # 8 groups per core

BF16 = ml_dtypes.bfloat16

_PROGRAM_CACHE = {}

NF_FOLD = int(os.environ.get("K_NF_FOLD", "4"))   # pairs with fused stationaries
# blocks whose relu+add is one DVE stt op (others: ACT Relu + DVE add)
BLOCKS_STT = tuple(
    int(x) for x in os.environ.get("K_BLOCKS_STT", "0,1").split(",") if x != "")
# blocks whose min() runs on Pool instead of DVE
BLOCKS_MIN_POOL = tuple(
    int(x) for x in os.environ.get("K_MIN_POOL", "").split(",") if x != "")
# which block slot of the NEXT group emits the previous group's PE tail
TAILPE_SLOT = int(os.environ.get("K_TAILPE", "2"))
# 1: emit prev group's mm3(3)+softmax AFTER the next head block (so the
# in-order PE/DVE streams run ready z/min work during the tail-chain wait)
MM3_LATE = int(os.environ.get("K_MM3LATE", "1"))
# how many block slots the score-reduce matmuls trail their block
MM3_DELAY = int(os.environ.get("K_MM3D", "2"))


def _build_program():
    import concourse.bass as bass
    import concourse.tile as tile
    from concourse import bacc, mybir

    f32 = mybir.dt.float32
    bf16 = mybir.dt.bfloat16
    AX = mybir.AxisListType
    OP = mybir.AluOpType
    AF = mybir.ActivationFunctionType

    nc = bacc.Bacc("TRN2", debug=False)

    # merged group images: HWDGE descriptor-gen is a serialized 625ns/DMA
    # shared device, so fewer/bigger DMAs win. Head image [q | ws | kT] is
    # consumed by the score blocks; kn0 stays separate (tail-lifetime);
    # kn1aux = [kn1 | m01(rows 0:32) | crow(rows 0:2)] on 72 partitions.
    BIGW = NPG + NF_FOLD * 128 + NPG * T
    OFF_WS = NPG
    OFF_KT = NPG + NF_FOLD * 128
    big_d = nc.dram_tensor("big", [G, 128, BIGW], bf16, kind="ExternalInput")
    kn0_d = nc.dram_tensor("kn0", [G, 128, NPG * 128], bf16, kind="ExternalInput")
    kn1_d = nc.dram_tensor("kn1", [G, 72, NPG * 128], bf16, kind="ExternalInput")
    # [m01 | NEXT group's crow(rows 0:2)] on 32 partitions
    AXW = T + (NPG // 2) * 128
    aux32_d = nc.dram_tensor("aux32", [G, 32, AXW], bf16, kind="ExternalInput")
    # packed constants: one bf16 image [bdwk|bdwp|w2rep|ones|crow(g=0)] and
    # one f32 image [id32|id64] — 2 DMAs instead of 7 at kernel start
    CBFW = 128 + 128 + NPG * GB + 2 * T + (NPG // 2) * 128
    cbf_d = nc.dram_tensor("cbf", [128, CBFW], bf16, kind="ExternalInput")
    cf32_d = nc.dram_tensor("cf32", [128, 96], f32, kind="ExternalInput")
    out_d = nc.dram_tensor("outp", [G, GB, E], f32, kind="ExternalOutput")

    with tile.TileContext(nc) as tc:
        with (
            tc.tile_pool(name="const", bufs=1) as cp,
            tc.tile_pool(name="gload", bufs=4) as gp,
            tc.tile_pool(name="qk", bufs=14) as qkp,
            tc.tile_pool(name="acts", bufs=6) as ap_,
            tc.tile_pool(name="sm", bufs=4) as smp,
            tc.tile_pool(name="zps", bufs=3, space=bass.MemorySpace.PSUM) as zp,
            tc.tile_pool(name="sps", bufs=2, space=bass.MemorySpace.PSUM) as sp,
        ):
            cbf = cp.tile([128, CBFW], bf16)
            # essentials for block (0,0) first; w2rep only needed MM3_DELAY
            # blocks later
            nc.sync.dma_start(cbf[:, 0:256], cbf_d[:, 0:256])
            nc.sync.dma_start(cbf[:, 256 + NPG * GB:],
                              cbf_d[:, 256 + NPG * GB:])
            nc.sync.dma_start(cbf[:, 256:256 + NPG * GB],
                              cbf_d[:, 256:256 + NPG * GB])
            cf32 = cp.tile([128, 96], f32)
            bdwk = cbf[:, 0:128]
            bdwp = cbf[:, 128:256]
            w2rep = cbf[:, 256:256 + NPG * GB]
            ones_r = cbf[0:2, 256 + NPG * GB:256 + NPG * GB + 2 * T]
            crow0 = cbf[0:2, 256 + NPG * GB + 2 * T:]
            id32 = cf32[0:32, 0:32]
            id64 = cf32[0:64, 32:96]

            # per-group state carried across the software pipeline
            gstate = {}

            def emit_dma_head(g):
                # one tall DMA per group: [q | ws | kT]
                bigg = gp.tile([128, BIGW], bf16, tag="bigg")
                if g == 0:
                    # fine-grained split so block-0 compute starts after the
                    # first piece (startup latency)
                    cuts = [0, OFF_KT + 4 * T, OFF_KT + 8 * T, BIGW]
                    for a, b in zip(cuts, cuts[1:]):
                        nc.sync.dma_start(bigg[:, a:b], big_d[g][:, a:b])
                else:
                    nc.sync.dma_start(bigg[:], big_d[g])
                # tensor_scalar requires an f32 scalar operand; up-convert
                # the bf16 q columns once per group
                qgf = gp.tile([128, NPG], f32, tag="qgf")
                nc.vector.tensor_copy(qgf[:], bigg[:, 0:NPG])
                gstate.setdefault(g, {}).update(
                    qg=qgf,
                    wsg=bigg[:, OFF_WS:OFF_KT],
                    kTg=bigg[:, OFF_KT:BIGW],
                )

            def emit_dma_tail(g):
                # weighted-sum tensors + mask, needed at the group tail.
                # The image also carries the NEXT group's c rows (crow) so
                # they arrive a full group before the score matmuls.
                kn0g = gp.tile([128, NPG * 128], bf16, tag="kn0g")
                nc.sync.dma_start(kn0g[:], kn0_d[g])
                kn1g = gp.tile([72, NPG * 128], bf16, tag="kn1g")
                nc.sync.dma_start(kn1g[:], kn1_d[g])
                axg = gp.tile([32, AXW], bf16, tag="axg")
                nc.sync.dma_start(axg[:], aux32_d[g])
                gstate[g].update(
                    kn0g=kn0g,
                    kn1g=kn1g,
                    m01g=axg[:, 0:T],
                )
                if g == 0:
                    gstate[g]["crowg"] = crow0
                if g + 1 <= G - 1:
                    gstate.setdefault(g + 1, {})["crowg"] = axg[0:2, T:]

            def emit_qk(g, jj):
                # q*k products for block jj on Pool, emitted a block early
                # so Pool runs ahead of the PE consumers
                st = gstate[g]
                for d2 in range(2):
                    j0 = 4 * jj + 2 * d2
                    if j0 + 1 < NF_FOLD:
                        continue
                    qk = qkp.tile([128, 2 * T], bf16, tag="qk")
                    nc.gpsimd.tensor_scalar_mul(
                        qk[:, 0:T], st["kTg"][:, j0 * T:(j0 + 1) * T],
                        st["qg"][:, j0:j0 + 1])
                    nc.gpsimd.tensor_scalar_mul(
                        qk[:, T:2 * T], st["kTg"][:, (j0 + 1) * T:(j0 + 2) * T],
                        st["qg"][:, j0 + 1:j0 + 2])
                    st[("qk", j0)] = qk

            def emit_block_head(g, jj, between=None):
                # one block = 4 pairs = two 2-pair duos at zsup cols 0 / 512
                # `between` runs after the matmuls but before the ACT/DVE
                # stage, so ready DVE work can fill the exp-latency window
                st = gstate[g]
                zsup = zp.tile([128, 1024], f32, tag="z")
                for d2 in range(2):
                    j0 = 4 * jj + 2 * d2          # first pair of the duo
                    zsl = zsup[:, 512 * d2: 512 * d2 + 2 * T]
                    if j0 + 1 < NF_FOLD:
                        # fused per-batch stationaries: c-broadcast first
                        # (zeroing the duo span), then one matmul per pair
                        nc.tensor.matmul(
                            zsl,
                            st["crowg"][0:2, (j0 // 2) * 128:(j0 // 2 + 1) * 128],
                            ones_r[:], start=True, stop=False,
                            skip_group_check=True,
                        )
                        for r in range(2):
                            j = j0 + r
                            nc.tensor.matmul(
                                zsup[:, 512 * d2 + r * T: 512 * d2 + (r + 1) * T],
                                st["wsg"][:, j * 128:(j + 1) * 128],
                                st["kTg"][:, j * T:(j + 1) * T],
                                start=False, stop=True, skip_group_check=True,
                            )
                        continue
                    ksl = st["kTg"][:, j0 * T:(j0 + 2) * T]        # [128, 400]
                    qk = st.pop(("qk", j0))
                    nc.tensor.matmul(zsl, bdwk[:], ksl,
                                     start=True, stop=False,
                                     skip_group_check=True)
                    nc.tensor.matmul(zsl, bdwp[:], qk[:],
                                     start=False, stop=False,
                                     skip_group_check=True)
                    nc.tensor.matmul(
                        zsl, st["crowg"][0:2, (j0 // 2) * 128:(j0 // 2 + 1) * 128],
                        ones_r[:], start=False, stop=True,
                        skip_group_check=True,
                    )
                if between is not None:
                    between()
                zv = zsup[:].rearrange("p (h c) -> p h c", h=2)[:, :, 0:2 * T]
                xsup = ap_.tile([128, 4 * T], bf16, tag="x")
                xv = xsup[:].rearrange("p (h c) -> p h c", h=2)
                nc.scalar.activation(xv, zv, AF.Exp)
                # elu(z)+1 == max(z,0) + min(exp(z),1) exactly; min runs in
                # DVE 4x mode (all-bf16 SBUF)
                xmsup = ap_.tile([128, 4 * T], bf16, tag="xm")
                min_eng = nc.gpsimd if jj in BLOCKS_MIN_POOL else nc.vector
                min_eng.tensor_scalar_min(xmsup[:], xsup[:], 1.0)
                xmv = xmsup[:].rearrange("p (h c) -> p h c", h=2)
                usup = ap_.tile([128, 4 * T], bf16, tag="ux")
                uv = usup[:].rearrange("p (h c) -> p h c", h=2)
                if jj in BLOCKS_STT:
                    nc.vector.scalar_tensor_tensor(
                        uv, zv, 0.0, xmv, op0=OP.max, op1=OP.add)
                else:
                    # relu on ACT (PSUM read), bf16 add on DVE (2x mode)
                    rsup = ap_.tile([128, 4 * T], bf16, tag="rx")
                    rv = rsup[:].rearrange("p (h c) -> p h c", h=2)
                    nc.scalar.activation(rv, zv, AF.Relu)
                    nc.vector.tensor_tensor(usup[:], rsup[:], xmsup[:],
                                            op=OP.add)
                st[("blk", jj)] = usup

            def emit_block_mm3(g, jj):
                st = gstate[g]
                usup = st.pop(("blk", jj))
                if "tail" not in st:
                    # one tail PSUM bank holds scores/eT/o4/fin per group
                    tail = sp.tile([128, 512], f32, tag="tail")
                    st["tail"] = tail
                scores_ps = st["tail"][0:GB, 0:T]
                for j4 in range(4):
                    j = 4 * jj + j4
                    w2sl = w2rep[:, j * GB:(j + 1) * GB]
                    nc.tensor.matmul(
                        scores_ps, w2sl, usup[:, j4 * T:(j4 + 1) * T],
                        start=(j == 0), stop=(j == NPG - 1),
                        skip_group_check=True,
                    )

            def emit_tail_sm(g):
                # softmax numerators (no max shift); mask-mult + row-sum in
                # one fused DVE op
                st = gstate[g]
                scores_ps = st["tail"][0:GB, 0:T]
                e_m = smp.tile([GB, T], f32, tag="em")
                nc.scalar.activation(e_m[:], scores_ps, AF.Exp)
                e_mm = smp.tile([GB, T], f32, tag="emm")
                rs = smp.tile([GB, 1], f32, tag="rs")
                nc.vector.tensor_tensor_reduce(
                    e_mm[:], e_m[:], st["m01g"][:], 1.0, 0.0,
                    OP.mult, OP.add, rs[:])
                ri = smp.tile([GB, 1], f32, tag="ri")
                nc.vector.reciprocal(ri[:], rs[:])
                st["e_mm"] = e_mm
                st["ri"] = ri

            def emit_tail_pe(g):
                st = gstate.pop(g)
                tail = st["tail"]
                e_mm, ri = st["e_mm"], st["ri"]
                eT0_ps = tail[:, 200:232]
                eT1_ps = tail[0:72, 232:264]
                o4 = tail[:, 264:296]
                fin_ps = tail[0:GB, 296:360]
                nc.tensor.transpose(eT0_ps, e_mm[:, 0:128], id32[:])
                nc.tensor.transpose(eT1_ps, e_mm[:, 128:200], id32[:])
                eT0 = smp.tile([128, 32], bf16, tag="eT0")
                nc.vector.tensor_copy(eT0[:], eT0_ps)
                eT1 = smp.tile([72, 32], bf16, tag="eT1")
                nc.vector.tensor_copy(eT1[:], eT1_ps)
                for j in range(NPG):
                    nc.tensor.matmul(
                        o4[:, 2 * j:2 * j + 2],
                        st["kn0g"][:, j * 128:(j + 1) * 128],
                        eT0[:, 2 * j:2 * j + 2], start=True, stop=False,
                        skip_group_check=True,
                    )
                    nc.tensor.matmul(
                        o4[:, 2 * j:2 * j + 2],
                        st["kn1g"][:, j * 128:(j + 1) * 128],
                        eT1[:, 2 * j:2 * j + 2], start=False, stop=True,
                        skip_group_check=True,
                    )
                osb = smp.tile([64, GB], f32, tag="osb")
                o4v = o4.rearrange("p (j two) -> p j two", two=2)
                osbv = osb[:].rearrange("p (j two) -> p j two", two=2)
                nc.vector.tensor_copy(osbv[:, :, 0:1], o4v[0:64, :, 0:1])
                nc.vector.tensor_copy(osbv[:, :, 1:2], o4v[64:128, :, 1:2])
                nc.tensor.transpose(fin_ps, osb[:], id64[:])
                fin = smp.tile([GB, 64], f32, tag="fins")
                nc.vector.tensor_scalar_mul(fin[:], fin_ps, ri[:])
                nc.sync.dma_start(out_d[g], fin[:])

            # software pipeline: mm3 deferred one block; tail spans groups;
            # DMA prefetched two groups ahead
            emit_dma_head(0)
            # identity matrices are tail-only; load them off the critical
            # startup path
            nc.sync.dma_start(cf32[:], cf32_d[:])
            emit_dma_head(1)
            emit_dma_tail(0)
            for jj in range(4):
                emit_qk(0, jj)
            for t in range(4 * G):
                g, jj = divmod(t, 4)
                if jj == 2:
                    if g + 2 < G:
                        emit_dma_head(g + 2)
                    if g + 1 < G:
                        emit_dma_tail(g + 1)
                emit_block_head(g, jj)
                # q*k products for the NEXT group, a full group early so
                # Pool output never gates this group's score matmuls
                if g + 1 < G:
                    emit_qk(g + 1, jj)
                # score-reduce trails MM3_DELAY blocks so the in-order PE
                # stream always has ready z-matmul work ahead of it
                if t - MM3_DELAY >= 0:
                    gm, jm = divmod(t - MM3_DELAY, 4)
                    emit_block_mm3(gm, jm)
                    if jm == 3:
                        emit_tail_sm(gm)
                gt = g - 1 - TAILPE_SLOT // 4
                if jj == TAILPE_SLOT % 4 and gt >= 0 and \
                        gt in gstate and "ri" in gstate[gt]:
                    emit_tail_pe(gt)
            for t in range(4 * G - MM3_DELAY, 4 * G):
                gm, jm = divmod(t, 4)
                emit_block_mm3(gm, jm)
                if jm == 3:
                    emit_tail_sm(gm)
            for g in range(G):
                if g in gstate and "ri" in gstate[g]:
                    emit_tail_pe(g)

    nc.compile()
    return nc


def _pack_inputs(queries, keys, mask, W1, b1, W2, b2):
    """Host-side packing into per-core input maps."""
    queries = np.asarray(queries, dtype=np.float32)
    keys = np.asarray(keys, dtype=np.float32)
    mask = np.asarray(mask)
    W1 = np.asarray(W1, dtype=np.float32)
    b1 = np.asarray(b1, dtype=np.float32)
    W2 = np.asarray(W2, dtype=np.float32)

    Wq = W1[0:E] + W1[2 * E:3 * E]        # query block + diff block
    Wk = W1[E:2 * E] - W1[2 * E:3 * E]    # key block - diff block
    Wp = W1[3 * E:4 * E]                  # product block

    # per-batch bias row c = q @ Wq + b1   -> (B, H)
    cvals = queries[:, 0, :] @ Wq + b1[None, :]

    # keys reshaped [core, group, pair, pb, t, e]
    K6 = keys.reshape(NCORES, G, NPG, 2, T, E)
    kT = np.ascontiguousarray(K6.transpose(0, 1, 3, 5, 2, 4)).reshape(
        NCORES, G, 128, NPG * T).astype(BF16)
    kn = np.ascontiguousarray(K6.transpose(0, 1, 4, 2, 3, 5)).reshape(
        NCORES, G, T, NPG * 128).astype(BF16)
    kn0 = np.ascontiguousarray(kn[:, :, :128])
    kn1 = np.ascontiguousarray(kn[:, :, 128:])

    Q5 = queries[:, 0, :].reshape(NCORES, G, NPG, 2, E)
    qp = np.ascontiguousarray(Q5.transpose(0, 1, 3, 4, 2)).reshape(
        NCORES, G, 128, NPG).astype(np.float32)

    # duo layout: row r of crow[g] holds pair (2*jj2 + r)'s c-row at free
    # offset jj2*128
    crow = np.ascontiguousarray(
        cvals.reshape(NCORES, G, NPG // 2, 2, 128).transpose(0, 1, 3, 2, 4)
    ).reshape(NCORES, G, 2, (NPG // 2) * 128).astype(BF16)

    # fused per-batch stationaries for the first NF_FOLD pairs per group:
    # Ws_b = Wk + diag(q_b) @ Wp, packed block-diagonal per pair
    ws = np.zeros((NCORES, G, 128, NF_FOLD * 128), np.float32)
    if NF_FOLD:
        Qf = Q5[:, :, :NF_FOLD]                            # (c, G, nf, 2, E)
        Wsb = Wk[None, None, None, None] + Qf[..., None] * Wp[None, None, None, None]
        for j in range(NF_FOLD):
            ws[:, :, 0:64, j * 128:j * 128 + 64] = Wsb[:, :, j, 0]
            ws[:, :, 64:128, j * 128 + 64:(j + 1) * 128] = Wsb[:, :, j, 1]
    ws = ws.astype(BF16)

    # tall head image per group: [q | ws | kT]
    big = np.concatenate([qp.astype(BF16), ws, kT], axis=3)

    # [m01 | NEXT group's crow(rows 0:2)] on 32 partitions
    m01 = mask.reshape(NCORES, G, GB, T).astype(BF16)
    crow32 = np.zeros((NCORES, G, 32, (NPG // 2) * 128), BF16)
    crow32[:, :G - 1, 0:2] = crow[:, 1:]
    aux32 = np.concatenate([m01, crow32], axis=3)

    bdwk = np.zeros((128, 128), np.float32)
    bdwk[0:64, 0:64] = Wk
    bdwk[64:128, 64:128] = Wk
    bdwp = np.zeros((128, 128), np.float32)
    bdwp[0:64, 0:64] = Wp
    bdwp[64:128, 64:128] = Wp

    w2rep = np.zeros((128, NPG * GB), np.float32)
    w2c = W2[:, 0]
    for j in range(NPG):
        w2rep[0:64, j * GB + 2 * j] = w2c
        w2rep[64:128, j * GB + 2 * j + 1] = w2c

    onesr = np.zeros((128, 2 * T), np.float32)
    onesr[0, 0:T] = 1.0
    onesr[1, T:2 * T] = 1.0
    cbf = np.concatenate(
        [bdwk, bdwp, w2rep, onesr], axis=1).astype(BF16)
    crow0c = np.zeros((NCORES, 128, (NPG // 2) * 128), BF16)
    crow0c[:, 0:2] = crow[:, 0]
    cbf = np.concatenate(
        [np.broadcast_to(cbf, (NCORES,) + cbf.shape), crow0c], axis=2)
    cf32 = np.zeros((128, 96), np.float32)
    cf32[0:32, 0:32] = np.eye(32)
    cf32[0:64, 32:96] = np.eye(64)

    in_maps = []
    for c in range(NCORES):
        m = {"big": big[c], "kn0": kn0[c], "kn1": kn1[c], "aux32": aux32[c],
             "cbf": np.ascontiguousarray(cbf[c]), "cf32": cf32}
        in_maps.append(m)
    return in_maps


def kernel(queries, keys, mask, W1, b1, W2, b2):
    from concourse import bass_utils

    key = "prog"
    if key not in _PROGRAM_CACHE:
        _PROGRAM_CACHE[key] = _build_program()
    nc = _PROGRAM_CACHE[key]

    in_maps = _pack_inputs(queries, keys, mask, W1, b1, W2, b2)
    res = bass_utils.run_bass_kernel_spmd(nc, in_maps, list(range(NCORES)))
    outs = [res.results[c]["outp"] for c in range(NCORES)]  # [G, GB, E] each
    out = np.stack(outs).reshape(B, E).astype(np.float32)
    return out[:, None, :]


if __name__ == "__main__":
    sys.path.insert(0, os.path.dirname(os.path.abspath(__file__)))
    import reference

    inputs = reference.setup_inputs()
    expected = np.asarray(reference.reference(**inputs))
    actual = kernel(**{k: np.asarray(v) for k, v in inputs.items()})
    err = np.abs(actual - expected).max()
    rel = err / max(1e-12, np.abs(expected).max())
    print("absmax err:", err, "rel:", rel)
